# revision 13
# baseline (speedup 1.0000x reference)
"""BEV->RV scatter-max kernel for 8 Trainium2 NeuronCores.

Strategy: shard by (batch, BEV-quadrant). Each BEV grid quadrant maps to a
disjoint RV column range (phi quadrants), so the 8 cores (2 batches x 4
quadrants) produce disjoint output slabs.

Layout (host, static/data-independent): pixels of each quadrant are grouped by
RV column into segments of <=SEG_K slots; segments are globally ordered by
their static row-window so each output row r only touches a contiguous hull of
segments (bounds are compile-time constants, union over quadrants). Per-pixel
static tables (row_low, and the 30-entry row_high profile H[z]) are
precomputed host-side with float32 arithmetic replicating the reference
bit-exactly.

Device: computes row_high by 30-plane select on z, then for each of the 64 RV
rows builds an additive {0,-1e30} mask and does 32 masked segmented
max-reduces (one per channel) over the row's segment hull. Host reduces
segments to columns and assembles the full output.
"""
import math
import sys

sys.path.insert(0, "/opt/trn_rl_repo")

import numpy as np

H_B, W_B = 512, 512
H_R, W_R = 64, 2048
Z_MIN, Z_MAX = -4.0, 2.0
Z_BINS = 30
Z_LOW = -1.73
PHI_MIN, PHI_MAX = -math.pi, math.pi
THETA_MIN, THETA_MAX = math.radians(-25.0), math.radians(3.0)
XMIN, XMAX, YMIN, YMAX = -50.0, 50.0, -50.0, 50.0

C = 32
B = 2
NEG = np.float32(-1.0e30)

SEG_K = 32          # slots per segment
SEG_PP = 19         # segments per partition
P = 128
F = SEG_K * SEG_PP  # free dim per partition
NSEG = P * SEG_PP   # segment capacity

_QUADS = {
    0: (slice(0, 256), slice(0, 256)),
    1: (slice(0, 256), slice(256, 512)),
    2: (slice(256, 512), slice(0, 256)),
    3: (slice(256, 512), slice(256, 512)),
}


def _geometry_f32():
    """Replicates reference._geometry() numpy-f32 semantics exactly."""
    y = np.linspace(YMAX, YMIN, H_B, dtype=np.float32)
    x = np.linspace(XMIN, XMAX, W_B, dtype=np.float32)
    yg, xg = np.meshgrid(y, x, indexing="ij")
    rho = np.sqrt((xg * xg + yg * yg).astype(np.float32)).astype(np.float32)
    phi = np.arctan2(yg, xg)
    theta_low = np.arctan2(np.float32(Z_LOW), rho)
    row_low = np.clip(
        np.rint((THETA_MAX - theta_low) / (THETA_MAX - THETA_MIN) * (H_R - 1)),
        0, H_R - 1,
    ).astype(np.int32)
    col = np.clip(
        np.rint((phi - PHI_MIN) / (PHI_MAX - PHI_MIN) * (W_R - 1)), 0, W_R - 1
    ).astype(np.int32)
    return rho, row_low, col


def _row_high_table(rho_flat):
    """H[z, n]: row_high for each z bin, f32 ops replicating the reference."""
    dz = (Z_MAX - Z_MIN) / Z_BINS
    zc = (np.arange(Z_BINS).astype(np.float32) * np.float32(dz)
          + np.float32(Z_MIN + dz / 2)).astype(np.float32)
    th = np.arctan2(zc[:, None].astype(np.float32), rho_flat[None, :]).astype(np.float32)
    a = (np.float32(THETA_MAX) - th).astype(np.float32)
    b = (a / np.float32(THETA_MAX - THETA_MIN)).astype(np.float32)
    cexpr = (b * np.float32(H_R - 1)).astype(np.float32)
    return np.clip(np.rint(cexpr), 0, H_R - 1).astype(np.int32)  # [30, N]


class _Static:
    pass


_S = None


def _build_static():
    global _S
    if _S is not None:
        return _S
    S = _Static()
    rho, row_low, col = _geometry_f32()
    S.quads = []
    hullA = np.full(H_R, NSEG, np.int64)
    hullB = np.full(H_R, -1, np.int64)
    for q in range(4):
        si, sj = _QUADS[q]
        qcol = col[si, sj].ravel()
        qrho = rho[si, sj].ravel().astype(np.float32)
        qrl = row_low[si, sj].ravel()
        ii, jj = np.meshgrid(np.arange(si.start, si.stop),
                             np.arange(sj.start, sj.stop), indexing="ij")
        qpix = (ii * W_B + jj).ravel()

        Hq = _row_high_table(qrho)
        smin_pix = np.minimum(qrl, Hq.min(0))
        smax_pix = np.maximum(qrl, Hq.max(0))

        # group pixels by column; within column order by row_low for tight
        # segment row-windows
        order = np.lexsort((qrl, qcol))
        c0, c1 = int(qcol.min()), int(qcol.max())
        ncols = c1 - c0 + 1
        counts = np.bincount(qcol - c0, minlength=ncols)

        seg_col, slot_src, seg_win = [], [], []
        pos = 0
        for ci in range(ncols):
            k = counts[ci]
            idxs = order[pos:pos + k]
            pos += k
            for off in range(0, k, SEG_K):
                chunk = idxs[off:off + SEG_K]
                seg_col.append(c0 + ci)
                slot_src.append(chunk)
                seg_win.append((smin_pix[chunk].min(), smax_pix[chunk].max()))
        nseg = len(seg_col)
        assert nseg <= NSEG, (q, nseg)
        seg_col = np.asarray(seg_col, np.int32)
        seg_win = np.asarray(seg_win, np.int64)

        # global segment order: by row-window center -> per-row contiguous hull
        gorder = np.argsort(seg_win[:, 0] + seg_win[:, 1], kind="stable")
        seg_col = seg_col[gorder]
        seg_win = seg_win[gorder]
        slot_src = [slot_src[s] for s in gorder]

        # per-row hulls (in global segment indices), accumulated over quadrants
        for r in range(H_R):
            act = np.flatnonzero((seg_win[:, 0] <= r) & (seg_win[:, 1] >= r))
            if act.size:
                hullA[r] = min(hullA[r], act.min())
                hullB[r] = max(hullB[r], act.max())

        dst_all, src_all = [], []
        for s, chunk in enumerate(slot_src):
            p_, j_ = s % P, s // P
            base = p_ * F + j_ * SEG_K
            dst_all.append(base + np.arange(len(chunk)))
        dst_all = np.concatenate(dst_all).astype(np.int64)
        src_all = np.concatenate([c for c in slot_src]).astype(np.int64)

        l_tab = np.full(P * F, 127.0, np.float32)
        l_tab[dst_all] = qrl[src_all].astype(np.float32)
        H_tab = np.full((Z_BINS, P * F), 127.0, np.float32)
        H_tab[:, dst_all] = Hq[:, src_all].astype(np.float32)

        Sq = _Static()
        Sq.c0, Sq.c1, Sq.ncols, Sq.nseg = c0, c1, ncols, nseg
        Sq.qpix_src = qpix[src_all]
        Sq.dst = dst_all
        Sq.l_tab = l_tab.reshape(P, F)
        Sq.H_tab = H_tab.reshape(Z_BINS, P, F)
        Sq.seg_col = seg_col
        # host reduction: reorder segments by column then reduceat
        Sq.col_order = np.argsort(seg_col, kind="stable")
        sc = seg_col[Sq.col_order]
        Sq.col_starts = np.flatnonzero(np.r_[True, sc[1:] != sc[:-1]])
        Sq.uniq_cols = sc[Sq.col_starts]
        S.quads.append(Sq)

    # quantize hulls to whole seg-slots (128 segments per slot j)
    S.hull_j = []
    for r in range(H_R):
        assert hullB[r] >= 0
        S.hull_j.append((int(hullA[r] // P), int(hullB[r] // P)))
    _S = S
    return S


_NC = None


def _build_nc():
    global _NC
    if _NC is not None:
        return _NC
    import concourse.bacc as bacc
    import concourse.mybir as mybir
    from concourse.tile import TileContext

    S = _build_static()
    nc = bacc.Bacc("TRN2", target_bir_lowering=False, debug=False, num_devices=8)
    vals = nc.declare_dram_parameter("vals", [C, P, F], mybir.dt.float32, isOutput=False)
    zb = nc.declare_dram_parameter("zb", [P, F], mybir.dt.float32, isOutput=False)
    ltab = nc.declare_dram_parameter("ltab", [P, F], mybir.dt.float32, isOutput=False)
    htab = nc.declare_dram_parameter("htab", [Z_BINS, P, F], mybir.dt.float32,
                                     isOutput=False)
    out = nc.declare_dram_parameter("out", [P, H_R * C * SEG_PP], mybir.dt.float32,
                                    isOutput=True)
    RH = H_R // 2

    with TileContext(nc) as tc:
        with tc.tile_pool(name="sb", bufs=1) as pool, \
             tc.tile_pool(name="hplane", bufs=2) as hpool:
            v_t = []
            for c in range(C):
                vt = pool.tile([P, F], mybir.dt.float32, tag=f"v{c}")
                nc.sync.dma_start(out=vt[:], in_=vals[c])
                v_t.append(vt)
            zb_t = pool.tile([P, F], mybir.dt.float32, tag="zb")
            l_t = pool.tile([P, F], mybir.dt.float32, tag="l")
            nc.sync.dma_start(out=zb_t[:], in_=zb[:, :])
            nc.sync.dma_start(out=l_t[:], in_=ltab[:, :])

            h_t = pool.tile([P, F], mybir.dt.float32, tag="h")
            eq_t = pool.tile([P, F], mybir.dt.float32, tag="eq")
            nc.vector.memset(h_t[:], 0.0)
            for z in range(Z_BINS):
                hp = hpool.tile([P, F], mybir.dt.float32, tag="hp")
                nc.sync.dma_start(out=hp[:], in_=htab[z])
                nc.vector.tensor_scalar(
                    out=eq_t[:], in0=zb_t[:], scalar1=float(z), scalar2=None,
                    op0=mybir.AluOpType.is_equal)
                nc.vector.tensor_tensor(
                    out=eq_t[:], in0=eq_t[:], in1=hp[:], op=mybir.AluOpType.mult)
                nc.vector.tensor_tensor(
                    out=h_t[:], in0=h_t[:], in1=eq_t[:], op=mybir.AluOpType.add)

            s_t = pool.tile([P, F], mybir.dt.float32, tag="s")
            e_t = pool.tile([P, F], mybir.dt.float32, tag="e")
            nc.vector.tensor_tensor(out=s_t[:], in0=l_t[:], in1=h_t[:],
                                    op=mybir.AluOpType.min)
            nc.vector.tensor_tensor(out=e_t[:], in0=l_t[:], in1=h_t[:],
                                    op=mybir.AluOpType.max)

            mask_t = pool.tile([P, F], mybir.dt.float32, tag="mask")
            mb_t = pool.tile([P, F], mybir.dt.float32, tag="mb")
            tmp_t = pool.tile([P, F], mybir.dt.float32, tag="tmp")
            for half in range(2):
                out_t = pool.tile([P, RH * C * SEG_PP], mybir.dt.float32,
                                  tag="out")
                nc.vector.memset(out_t[:], float(NEG))
                for r in range(half * RH, (half + 1) * RH):
                    fr = float(r)
                    jA, jB = S.hull_j[r]
                    lo, hi = jA * SEG_K, (jB + 1) * SEG_K
                    nc.vector.tensor_scalar(
                        out=mask_t[:, lo:hi], in0=s_t[:, lo:hi], scalar1=fr,
                        scalar2=None, op0=mybir.AluOpType.is_le)
                    nc.vector.tensor_scalar(
                        out=mb_t[:, lo:hi], in0=e_t[:, lo:hi], scalar1=fr,
                        scalar2=None, op0=mybir.AluOpType.is_ge)
                    nc.vector.tensor_tensor(
                        out=mask_t[:, lo:hi], in0=mask_t[:, lo:hi],
                        in1=mb_t[:, lo:hi], op=mybir.AluOpType.mult)
                    nc.vector.tensor_scalar(
                        out=mb_t[:, lo:hi], in0=mask_t[:, lo:hi],
                        scalar1=float(1.0e30), scalar2=float(-1.0e30),
                        op0=mybir.AluOpType.mult, op1=mybir.AluOpType.add)
                    for c in range(C):
                        nc.vector.tensor_tensor(
                            out=tmp_t[:, lo:hi], in0=v_t[c][:, lo:hi],
                            in1=mb_t[:, lo:hi], op=mybir.AluOpType.add)
                        off = ((r - half * RH) * C + c) * SEG_PP
                        nc.vector.tensor_reduce(
                            out=out_t[:, off + jA:off + jB + 1],
                            in_=tmp_t[:, lo:hi].rearrange("p (j k) -> p j k",
                                                          k=SEG_K),
                            axis=mybir.AxisListType.X,
                            op=mybir.AluOpType.max)
                nc.sync.dma_start(
                    out=out[:, half * RH * C * SEG_PP:(half + 1) * RH * C * SEG_PP],
                    in_=out_t[:])
    nc.compile()
    _NC = nc
    return nc


def kernel(bev_feat, bev_z_bin):
    from concourse.bass_utils import run_bass_kernel_spmd

    S = _build_static()
    nc = _build_nc()
    bev_feat = np.asarray(bev_feat, dtype=np.float32)
    bev_z_bin = np.asarray(bev_z_bin, dtype=np.int32)

    in_maps = []
    metas = []
    for core in range(8):
        b, q = core // 4, core % 4
        Sq = S.quads[q]
        flat = bev_feat[b].reshape(C, H_B * W_B)
        v = np.full((C, P * F), NEG, np.float32)
        v[:, Sq.dst] = flat[:, Sq.qpix_src]
        zflat = bev_z_bin[b, 0].reshape(H_B * W_B)
        z = np.zeros(P * F, np.float32)
        z[Sq.dst] = zflat[Sq.qpix_src].astype(np.float32)
        in_maps.append({
            "vals": v.reshape(C, P, F),
            "zb": z.reshape(P, F),
            "ltab": Sq.l_tab,
            "htab": Sq.H_tab,
        })
        metas.append((b, q))

    res = run_bass_kernel_spmd(nc, in_maps, list(range(8)))

    outp = np.zeros((B, C, H_R, W_R), np.float32)
    for core, (b, q) in enumerate(metas):
        Sq = S.quads[q]
        o = res.results[core]["out"].reshape(P, H_R, C, SEG_PP)
        # segment s lives at partition s % P, seg-slot s // P
        o = o.transpose(1, 2, 3, 0).reshape(H_R, C, NSEG)[:, :, :Sq.nseg]
        o = o[:, :, Sq.col_order]
        red = np.maximum.reduceat(o, Sq.col_starts, axis=2)
        block = np.where(red < -1.0e29, np.float32(0), red)
        # block is [H_R, C, ncols_used] -> outp[b] is [C, H_R, W_R]
        outp[b][:, :, Sq.uniq_cols] = block.transpose(1, 0, 2)
    return outp


# revision 15
# speedup vs baseline: 5071.9309x; 5071.9309x over previous
"""BEV->RV scatter-max kernel for 8 Trainium2 NeuronCores.

Strategy: shard by (batch, BEV-quadrant). Each BEV grid quadrant maps to a
disjoint RV column range (phi quadrants), so the 8 cores (2 batches x 4
quadrants) produce disjoint output slabs.

Layout (host, static/data-independent): pixels of each quadrant are grouped by
RV column into segments of <=SEG_K slots; segments are globally ordered by
their static row-window so each output row r only touches a contiguous hull of
segments (bounds are compile-time constants, union over quadrants). Per-pixel
static tables (row_low, and the 30-entry row_high profile H[z]) are
precomputed host-side with float32 arithmetic replicating the reference
bit-exactly.

Device: computes row_high by 30-plane select on z, then for each of the 64 RV
rows builds an additive {0,-1e30} mask and does 32 masked segmented
max-reduces (one per channel) over the row's segment hull. Host reduces
segments to columns and assembles the full output.
"""
import math
import sys

sys.path.insert(0, "/opt/trn_rl_repo")

import numpy as np

H_B, W_B = 512, 512
H_R, W_R = 64, 2048
Z_MIN, Z_MAX = -4.0, 2.0
Z_BINS = 30
Z_LOW = -1.73
PHI_MIN, PHI_MAX = -math.pi, math.pi
THETA_MIN, THETA_MAX = math.radians(-25.0), math.radians(3.0)
XMIN, XMAX, YMIN, YMAX = -50.0, 50.0, -50.0, 50.0

C = 32
B = 2
NEG = np.float32(-1.0e30)

_REPS = 1           # timing instrumentation: repeat the device main loop
SEG_K = 32          # slots per segment
SEG_PP = 19         # segments per partition
P = 128
F = SEG_K * SEG_PP  # free dim per partition
NSEG = P * SEG_PP   # segment capacity

_QUADS = {
    0: (slice(0, 256), slice(0, 256)),
    1: (slice(0, 256), slice(256, 512)),
    2: (slice(256, 512), slice(0, 256)),
    3: (slice(256, 512), slice(256, 512)),
}


def _geometry_f32():
    """Replicates reference._geometry() numpy-f32 semantics exactly."""
    y = np.linspace(YMAX, YMIN, H_B, dtype=np.float32)
    x = np.linspace(XMIN, XMAX, W_B, dtype=np.float32)
    yg, xg = np.meshgrid(y, x, indexing="ij")
    rho = np.sqrt((xg * xg + yg * yg).astype(np.float32)).astype(np.float32)
    phi = np.arctan2(yg, xg)
    theta_low = np.arctan2(np.float32(Z_LOW), rho)
    row_low = np.clip(
        np.rint((THETA_MAX - theta_low) / (THETA_MAX - THETA_MIN) * (H_R - 1)),
        0, H_R - 1,
    ).astype(np.int32)
    col = np.clip(
        np.rint((phi - PHI_MIN) / (PHI_MAX - PHI_MIN) * (W_R - 1)), 0, W_R - 1
    ).astype(np.int32)
    return rho, row_low, col


def _row_high_table(rho_flat):
    """H[z, n]: row_high for each z bin, f32 ops replicating the reference."""
    dz = (Z_MAX - Z_MIN) / Z_BINS
    zc = (np.arange(Z_BINS).astype(np.float32) * np.float32(dz)
          + np.float32(Z_MIN + dz / 2)).astype(np.float32)
    th = np.arctan2(zc[:, None].astype(np.float32), rho_flat[None, :]).astype(np.float32)
    a = (np.float32(THETA_MAX) - th).astype(np.float32)
    b = (a / np.float32(THETA_MAX - THETA_MIN)).astype(np.float32)
    cexpr = (b * np.float32(H_R - 1)).astype(np.float32)
    return np.clip(np.rint(cexpr), 0, H_R - 1).astype(np.int32)  # [30, N]


class _Static:
    pass


_S = None


def _build_static():
    global _S
    if _S is not None:
        return _S
    S = _Static()
    rho, row_low, col = _geometry_f32()
    S.quads = []
    hullA = np.full(H_R, NSEG, np.int64)
    hullB = np.full(H_R, -1, np.int64)
    for q in range(4):
        si, sj = _QUADS[q]
        qcol = col[si, sj].ravel()
        qrho = rho[si, sj].ravel().astype(np.float32)
        qrl = row_low[si, sj].ravel()
        ii, jj = np.meshgrid(np.arange(si.start, si.stop),
                             np.arange(sj.start, sj.stop), indexing="ij")
        qpix = (ii * W_B + jj).ravel()

        Hq = _row_high_table(qrho)
        smin_pix = np.minimum(qrl, Hq.min(0))
        smax_pix = np.maximum(qrl, Hq.max(0))

        # group pixels by column; within column order by row_low for tight
        # segment row-windows
        order = np.lexsort((qrl, qcol))
        c0, c1 = int(qcol.min()), int(qcol.max())
        ncols = c1 - c0 + 1
        counts = np.bincount(qcol - c0, minlength=ncols)

        seg_col, slot_src, seg_win = [], [], []
        pos = 0
        for ci in range(ncols):
            k = counts[ci]
            idxs = order[pos:pos + k]
            pos += k
            for off in range(0, k, SEG_K):
                chunk = idxs[off:off + SEG_K]
                seg_col.append(c0 + ci)
                slot_src.append(chunk)
                seg_win.append((smin_pix[chunk].min(), smax_pix[chunk].max()))
        nseg = len(seg_col)
        assert nseg <= NSEG, (q, nseg)
        seg_col = np.asarray(seg_col, np.int32)
        seg_win = np.asarray(seg_win, np.int64)

        # global segment order: by row-window center -> per-row contiguous hull
        gorder = np.argsort(seg_win[:, 0] + seg_win[:, 1], kind="stable")
        seg_col = seg_col[gorder]
        seg_win = seg_win[gorder]
        slot_src = [slot_src[s] for s in gorder]

        # per-row hulls (in global segment indices), accumulated over quadrants
        for r in range(H_R):
            act = np.flatnonzero((seg_win[:, 0] <= r) & (seg_win[:, 1] >= r))
            if act.size:
                hullA[r] = min(hullA[r], act.min())
                hullB[r] = max(hullB[r], act.max())

        dst_all, src_all = [], []
        for s, chunk in enumerate(slot_src):
            p_, j_ = s % P, s // P
            base = p_ * F + j_ * SEG_K
            dst_all.append(base + np.arange(len(chunk)))
        dst_all = np.concatenate(dst_all).astype(np.int64)
        src_all = np.concatenate([c for c in slot_src]).astype(np.int64)

        l_tab = np.full(P * F, 127.0, np.float32)
        l_tab[dst_all] = qrl[src_all].astype(np.float32)
        H_tab = np.full((Z_BINS, P * F), 127.0, np.float32)
        H_tab[:, dst_all] = Hq[:, src_all].astype(np.float32)

        Sq = _Static()
        Sq.c0, Sq.c1, Sq.ncols, Sq.nseg = c0, c1, ncols, nseg
        Sq.qpix_src = qpix[src_all]
        Sq.dst = dst_all
        Sq.l_tab = l_tab.reshape(P, F)
        Sq.H_tab = H_tab.reshape(Z_BINS, P, F)
        Sq.seg_col = seg_col
        # host reduction: reorder segments by column then reduceat
        Sq.col_order = np.argsort(seg_col, kind="stable")
        sc = seg_col[Sq.col_order]
        Sq.col_starts = np.flatnonzero(np.r_[True, sc[1:] != sc[:-1]])
        Sq.uniq_cols = sc[Sq.col_starts]
        S.quads.append(Sq)

    # quantize hulls to whole seg-slots (128 segments per slot j)
    S.hull_j = []
    for r in range(H_R):
        assert hullB[r] >= 0
        S.hull_j.append((int(hullA[r] // P), int(hullB[r] // P)))
    _S = S
    return S


_NC = None


def _build_nc():
    global _NC
    if _NC is not None:
        return _NC
    import concourse.bacc as bacc
    import concourse.mybir as mybir
    from concourse.tile import TileContext

    S = _build_static()
    nc = bacc.Bacc("TRN2", target_bir_lowering=False, debug=False, num_devices=8)
    vals = nc.declare_dram_parameter("vals", [C, P, F], mybir.dt.float32, isOutput=False)
    zb = nc.declare_dram_parameter("zb", [P, F], mybir.dt.float32, isOutput=False)
    ltab = nc.declare_dram_parameter("ltab", [P, F], mybir.dt.float32, isOutput=False)
    htab = nc.declare_dram_parameter("htab", [Z_BINS, P, F], mybir.dt.float32,
                                     isOutput=False)
    out = nc.declare_dram_parameter("out", [P, H_R * C * SEG_PP], mybir.dt.float32,
                                    isOutput=True)
    RH = H_R // 2

    with TileContext(nc) as tc:
        with tc.tile_pool(name="sb", bufs=1) as pool, \
             tc.tile_pool(name="hplane", bufs=2) as hpool:
            v_t = []
            for c in range(C):
                vt = pool.tile([P, F], mybir.dt.float32, tag=f"v{c}")
                nc.sync.dma_start(out=vt[:], in_=vals[c])
                v_t.append(vt)
            zb_t = pool.tile([P, F], mybir.dt.float32, tag="zb")
            l_t = pool.tile([P, F], mybir.dt.float32, tag="l")
            nc.sync.dma_start(out=zb_t[:], in_=zb[:, :])
            nc.sync.dma_start(out=l_t[:], in_=ltab[:, :])

            h_t = pool.tile([P, F], mybir.dt.float32, tag="h")
            eq_t = pool.tile([P, F], mybir.dt.float32, tag="eq")
            nc.vector.memset(h_t[:], 0.0)
            for z in range(Z_BINS):
                hp = hpool.tile([P, F], mybir.dt.float32, tag="hp")
                nc.sync.dma_start(out=hp[:], in_=htab[z])
                nc.vector.tensor_scalar(
                    out=eq_t[:], in0=zb_t[:], scalar1=float(z), scalar2=None,
                    op0=mybir.AluOpType.is_equal)
                nc.vector.tensor_tensor(
                    out=eq_t[:], in0=eq_t[:], in1=hp[:], op=mybir.AluOpType.mult)
                nc.vector.tensor_tensor(
                    out=h_t[:], in0=h_t[:], in1=eq_t[:], op=mybir.AluOpType.add)

            s_t = pool.tile([P, F], mybir.dt.float32, tag="s")
            e_t = pool.tile([P, F], mybir.dt.float32, tag="e")
            nc.vector.tensor_tensor(out=s_t[:], in0=l_t[:], in1=h_t[:],
                                    op=mybir.AluOpType.min)
            nc.vector.tensor_tensor(out=e_t[:], in0=l_t[:], in1=h_t[:],
                                    op=mybir.AluOpType.max)

            mask_t = pool.tile([P, F], mybir.dt.float32, tag="mask")
            mb_t = pool.tile([P, F], mybir.dt.float32, tag="mb")
            tmp_t = pool.tile([P, F], mybir.dt.float32, tag="tmp")
            for _rep in range(_REPS):
              for half in range(2):
                out_t = pool.tile([P, RH * C * SEG_PP], mybir.dt.float32,
                                  tag="out")
                nc.vector.memset(out_t[:], float(NEG))
                for r in range(half * RH, (half + 1) * RH):
                    fr = float(r)
                    jA, jB = S.hull_j[r]
                    lo, hi = jA * SEG_K, (jB + 1) * SEG_K
                    nc.vector.tensor_scalar(
                        out=mask_t[:, lo:hi], in0=s_t[:, lo:hi], scalar1=fr,
                        scalar2=None, op0=mybir.AluOpType.is_le)
                    nc.vector.tensor_scalar(
                        out=mb_t[:, lo:hi], in0=e_t[:, lo:hi], scalar1=fr,
                        scalar2=None, op0=mybir.AluOpType.is_ge)
                    nc.vector.tensor_tensor(
                        out=mask_t[:, lo:hi], in0=mask_t[:, lo:hi],
                        in1=mb_t[:, lo:hi], op=mybir.AluOpType.mult)
                    nc.vector.tensor_scalar(
                        out=mb_t[:, lo:hi], in0=mask_t[:, lo:hi],
                        scalar1=float(1.0e30), scalar2=float(-1.0e30),
                        op0=mybir.AluOpType.mult, op1=mybir.AluOpType.add)
                    for c in range(C):
                        nc.vector.tensor_tensor(
                            out=tmp_t[:, lo:hi], in0=v_t[c][:, lo:hi],
                            in1=mb_t[:, lo:hi], op=mybir.AluOpType.add)
                        off = ((r - half * RH) * C + c) * SEG_PP
                        nc.vector.tensor_reduce(
                            out=out_t[:, off + jA:off + jB + 1],
                            in_=tmp_t[:, lo:hi].rearrange("p (j k) -> p j k",
                                                          k=SEG_K),
                            axis=mybir.AxisListType.X,
                            op=mybir.AluOpType.max)
                nc.sync.dma_start(
                    out=out[:, half * RH * C * SEG_PP:(half + 1) * RH * C * SEG_PP],
                    in_=out_t[:])
    nc.compile()
    _NC = nc
    return nc


def kernel(bev_feat, bev_z_bin):
    from concourse.bass_utils import run_bass_kernel_spmd

    S = _build_static()
    nc = _build_nc()
    bev_feat = np.asarray(bev_feat, dtype=np.float32)
    bev_z_bin = np.asarray(bev_z_bin, dtype=np.int32)

    in_maps = []
    metas = []
    for core in range(8):
        b, q = core // 4, core % 4
        Sq = S.quads[q]
        flat = bev_feat[b].reshape(C, H_B * W_B)
        v = np.full((C, P * F), NEG, np.float32)
        v[:, Sq.dst] = flat[:, Sq.qpix_src]
        zflat = bev_z_bin[b, 0].reshape(H_B * W_B)
        z = np.zeros(P * F, np.float32)
        z[Sq.dst] = zflat[Sq.qpix_src].astype(np.float32)
        in_maps.append({
            "vals": v.reshape(C, P, F),
            "zb": z.reshape(P, F),
            "ltab": Sq.l_tab,
            "htab": Sq.H_tab,
        })
        metas.append((b, q))

    res = run_bass_kernel_spmd(nc, in_maps, list(range(8)))

    outp = np.zeros((B, C, H_R, W_R), np.float32)
    for core, (b, q) in enumerate(metas):
        Sq = S.quads[q]
        o = res.results[core]["out"].reshape(P, H_R, C, SEG_PP)
        # segment s lives at partition s % P, seg-slot s // P
        o = o.transpose(1, 2, 3, 0).reshape(H_R, C, NSEG)[:, :, :Sq.nseg]
        o = o[:, :, Sq.col_order]
        red = np.maximum.reduceat(o, Sq.col_starts, axis=2)
        block = np.where(red < -1.0e29, np.float32(0), red)
        # block is [H_R, C, ncols_used] -> outp[b] is [C, H_R, W_R]
        outp[b][:, :, Sq.uniq_cols] = block.transpose(1, 0, 2)
    return outp


# revision 17
# speedup vs baseline: 5698.7146x; 1.1236x over previous
"""BEV->RV scatter-max kernel for 8 Trainium2 NeuronCores.

Strategy: shard by (batch, BEV-quadrant). Each BEV grid quadrant maps to a
disjoint RV column range (phi quadrants), so the 8 cores (2 batches x 4
quadrants) produce disjoint output slabs.

Layout (host, static/data-independent): pixels of each quadrant are grouped by
RV column into segments of <=SEG_K slots; segments are globally ordered by
their static row-window so each output row r only touches a contiguous hull of
segments (bounds are compile-time constants, union over quadrants). Per-pixel
static tables (row_low, and the 30-entry row_high profile H[z]) are
precomputed host-side with float32 arithmetic replicating the reference
bit-exactly.

Device: computes row_high by 30-plane select on z, then for each of the 64 RV
rows builds an additive {0,-1e30} mask and does 32 masked segmented
max-reduces (one per channel) over the row's segment hull. Host reduces
segments to columns and assembles the full output.
"""
import math
import sys

sys.path.insert(0, "/opt/trn_rl_repo")

import numpy as np

H_B, W_B = 512, 512
H_R, W_R = 64, 2048
Z_MIN, Z_MAX = -4.0, 2.0
Z_BINS = 30
Z_LOW = -1.73
PHI_MIN, PHI_MAX = -math.pi, math.pi
THETA_MIN, THETA_MAX = math.radians(-25.0), math.radians(3.0)
XMIN, XMAX, YMIN, YMAX = -50.0, 50.0, -50.0, 50.0

C = 32
B = 2
NEG = np.float32(-1.0e30)

_REPS = 1           # timing instrumentation: repeat the device main loop
SEG_K = 32          # slots per segment
SEG_PP = 19         # segments per partition
P = 128
F = SEG_K * SEG_PP  # free dim per partition
NSEG = P * SEG_PP   # segment capacity

_QUADS = {
    0: (slice(0, 256), slice(0, 256)),
    1: (slice(0, 256), slice(256, 512)),
    2: (slice(256, 512), slice(0, 256)),
    3: (slice(256, 512), slice(256, 512)),
}


def _geometry_f32():
    """Replicates reference._geometry() numpy-f32 semantics exactly."""
    y = np.linspace(YMAX, YMIN, H_B, dtype=np.float32)
    x = np.linspace(XMIN, XMAX, W_B, dtype=np.float32)
    yg, xg = np.meshgrid(y, x, indexing="ij")
    rho = np.sqrt((xg * xg + yg * yg).astype(np.float32)).astype(np.float32)
    phi = np.arctan2(yg, xg)
    theta_low = np.arctan2(np.float32(Z_LOW), rho)
    row_low = np.clip(
        np.rint((THETA_MAX - theta_low) / (THETA_MAX - THETA_MIN) * (H_R - 1)),
        0, H_R - 1,
    ).astype(np.int32)
    col = np.clip(
        np.rint((phi - PHI_MIN) / (PHI_MAX - PHI_MIN) * (W_R - 1)), 0, W_R - 1
    ).astype(np.int32)
    return rho, row_low, col


def _row_high_table(rho_flat):
    """H[z, n]: row_high for each z bin, f32 ops replicating the reference."""
    dz = (Z_MAX - Z_MIN) / Z_BINS
    zc = (np.arange(Z_BINS).astype(np.float32) * np.float32(dz)
          + np.float32(Z_MIN + dz / 2)).astype(np.float32)
    th = np.arctan2(zc[:, None].astype(np.float32), rho_flat[None, :]).astype(np.float32)
    a = (np.float32(THETA_MAX) - th).astype(np.float32)
    b = (a / np.float32(THETA_MAX - THETA_MIN)).astype(np.float32)
    cexpr = (b * np.float32(H_R - 1)).astype(np.float32)
    return np.clip(np.rint(cexpr), 0, H_R - 1).astype(np.int32)  # [30, N]


class _Static:
    pass


_S = None


def _build_static():
    global _S
    if _S is not None:
        return _S
    S = _Static()
    rho, row_low, col = _geometry_f32()
    S.quads = []
    hullA = np.full(H_R, NSEG, np.int64)
    hullB = np.full(H_R, -1, np.int64)
    for q in range(4):
        si, sj = _QUADS[q]
        qcol = col[si, sj].ravel()
        qrho = rho[si, sj].ravel().astype(np.float32)
        qrl = row_low[si, sj].ravel()
        ii, jj = np.meshgrid(np.arange(si.start, si.stop),
                             np.arange(sj.start, sj.stop), indexing="ij")
        qpix = (ii * W_B + jj).ravel()

        Hq = _row_high_table(qrho)
        smin_pix = np.minimum(qrl, Hq.min(0))
        smax_pix = np.maximum(qrl, Hq.max(0))

        # group pixels by column; within column order by row_low for tight
        # segment row-windows
        order = np.lexsort((qrl, qcol))
        c0, c1 = int(qcol.min()), int(qcol.max())
        ncols = c1 - c0 + 1
        counts = np.bincount(qcol - c0, minlength=ncols)

        seg_col, slot_src, seg_win = [], [], []
        pos = 0
        for ci in range(ncols):
            k = counts[ci]
            idxs = order[pos:pos + k]
            pos += k
            for off in range(0, k, SEG_K):
                chunk = idxs[off:off + SEG_K]
                seg_col.append(c0 + ci)
                slot_src.append(chunk)
                seg_win.append((smin_pix[chunk].min(), smax_pix[chunk].max()))
        nseg = len(seg_col)
        assert nseg <= NSEG, (q, nseg)
        seg_col = np.asarray(seg_col, np.int32)
        seg_win = np.asarray(seg_win, np.int64)

        # global segment order: by row-window center -> per-row contiguous hull
        gorder = np.argsort(seg_win[:, 0] + seg_win[:, 1], kind="stable")
        seg_col = seg_col[gorder]
        seg_win = seg_win[gorder]
        slot_src = [slot_src[s] for s in gorder]

        # per-row hulls (in global segment indices), accumulated over quadrants
        for r in range(H_R):
            act = np.flatnonzero((seg_win[:, 0] <= r) & (seg_win[:, 1] >= r))
            if act.size:
                hullA[r] = min(hullA[r], act.min())
                hullB[r] = max(hullB[r], act.max())

        dst_all, src_all = [], []
        for s, chunk in enumerate(slot_src):
            p_, j_ = s % P, s // P
            base = p_ * F + j_ * SEG_K
            dst_all.append(base + np.arange(len(chunk)))
        dst_all = np.concatenate(dst_all).astype(np.int64)
        src_all = np.concatenate([c for c in slot_src]).astype(np.int64)

        l_tab = np.full(P * F, 127.0, np.float32)
        l_tab[dst_all] = qrl[src_all].astype(np.float32)
        H_tab = np.full((Z_BINS, P * F), 127.0, np.float32)
        H_tab[:, dst_all] = Hq[:, src_all].astype(np.float32)

        Sq = _Static()
        Sq.c0, Sq.c1, Sq.ncols, Sq.nseg = c0, c1, ncols, nseg
        Sq.qpix_src = qpix[src_all]
        Sq.dst = dst_all
        Sq.l_tab = l_tab.reshape(P, F)
        Sq.H_tab = H_tab.reshape(Z_BINS, P, F)
        Sq.seg_col = seg_col
        # host reduction: reorder segments by column then reduceat
        Sq.col_order = np.argsort(seg_col, kind="stable")
        sc = seg_col[Sq.col_order]
        Sq.col_starts = np.flatnonzero(np.r_[True, sc[1:] != sc[:-1]])
        Sq.uniq_cols = sc[Sq.col_starts]
        S.quads.append(Sq)

    # quantize hulls to whole seg-slots (128 segments per slot j)
    S.hull_j = []
    for r in range(H_R):
        assert hullB[r] >= 0
        S.hull_j.append((int(hullA[r] // P), int(hullB[r] // P)))
    _S = S
    return S


_NC = None


def _build_nc():
    global _NC
    if _NC is not None:
        return _NC
    import concourse.bacc as bacc
    import concourse.mybir as mybir
    from concourse.tile import TileContext

    S = _build_static()
    nc = bacc.Bacc("TRN2", target_bir_lowering=False, debug=False, num_devices=8)
    vals = nc.declare_dram_parameter("vals", [C, P, F], mybir.dt.float32, isOutput=False)
    zb = nc.declare_dram_parameter("zb", [P, F], mybir.dt.float32, isOutput=False)
    ltab = nc.declare_dram_parameter("ltab", [P, F], mybir.dt.float32, isOutput=False)
    htab = nc.declare_dram_parameter("htab", [Z_BINS, P, F], mybir.dt.float32,
                                     isOutput=False)
    out = nc.declare_dram_parameter("out", [P, H_R * C * SEG_PP], mybir.dt.float32,
                                    isOutput=True)
    RH = H_R // 2

    with TileContext(nc) as tc:
        with tc.tile_pool(name="sb", bufs=1) as pool, \
             tc.tile_pool(name="hplane", bufs=2) as hpool:
            v_t = []
            for c in range(C):
                vt = pool.tile([P, F], mybir.dt.float32, tag=f"v{c}")
                nc.sync.dma_start(out=vt[:], in_=vals[c])
                v_t.append(vt)
            zb_t = pool.tile([P, F], mybir.dt.float32, tag="zb")
            l_t = pool.tile([P, F], mybir.dt.float32, tag="l")
            nc.sync.dma_start(out=zb_t[:], in_=zb[:, :])
            nc.sync.dma_start(out=l_t[:], in_=ltab[:, :])

            h_t = pool.tile([P, F], mybir.dt.float32, tag="h")
            eq_t = pool.tile([P, F], mybir.dt.float32, tag="eq")
            nc.vector.memset(h_t[:], 0.0)
            for z in range(Z_BINS):
                hp = hpool.tile([P, F], mybir.dt.float32, tag="hp")
                nc.sync.dma_start(out=hp[:], in_=htab[z])
                nc.vector.tensor_scalar(
                    out=eq_t[:], in0=zb_t[:], scalar1=float(z), scalar2=None,
                    op0=mybir.AluOpType.is_equal)
                nc.vector.tensor_tensor(
                    out=eq_t[:], in0=eq_t[:], in1=hp[:], op=mybir.AluOpType.mult)
                nc.vector.tensor_tensor(
                    out=h_t[:], in0=h_t[:], in1=eq_t[:], op=mybir.AluOpType.add)

            s_t = pool.tile([P, F], mybir.dt.float32, tag="s")
            e_t = pool.tile([P, F], mybir.dt.float32, tag="e")
            nc.vector.tensor_tensor(out=s_t[:], in0=l_t[:], in1=h_t[:],
                                    op=mybir.AluOpType.min)
            nc.vector.tensor_tensor(out=e_t[:], in0=l_t[:], in1=h_t[:],
                                    op=mybir.AluOpType.max)

            mask_t = pool.tile([P, F], mybir.dt.float32, tag="mask")
            mb_t = pool.tile([P, F], mybir.dt.float32, tag="mb")
            tmp_t = pool.tile([P, F], mybir.dt.float32, tag="tmp")
            tmpg_t = pool.tile([P, F], mybir.dt.float32, tag="tmpg")
            for _rep in range(_REPS):
              for half in range(2):
                out_t = pool.tile([P, RH * C * SEG_PP], mybir.dt.float32,
                                  tag="out")
                nc.vector.memset(out_t[:], float(NEG))
                for r in range(half * RH, (half + 1) * RH):
                    fr = float(r)
                    jA, jB = S.hull_j[r]
                    lo, hi = jA * SEG_K, (jB + 1) * SEG_K
                    nc.vector.tensor_scalar(
                        out=mask_t[:, lo:hi], in0=s_t[:, lo:hi], scalar1=fr,
                        scalar2=None, op0=mybir.AluOpType.is_le)
                    nc.vector.tensor_scalar(
                        out=mb_t[:, lo:hi], in0=e_t[:, lo:hi], scalar1=fr,
                        scalar2=None, op0=mybir.AluOpType.is_ge)
                    nc.vector.tensor_tensor(
                        out=mask_t[:, lo:hi], in0=mask_t[:, lo:hi],
                        in1=mb_t[:, lo:hi], op=mybir.AluOpType.mult)
                    nc.vector.tensor_scalar(
                        out=mb_t[:, lo:hi], in0=mask_t[:, lo:hi],
                        scalar1=float(1.0e30), scalar2=float(-1.0e30),
                        op0=mybir.AluOpType.mult, op1=mybir.AluOpType.add)
                    for c in range(C):
                        # alternate the add between GpSimd and DVE so the adds
                        # overlap the DVE reduces
                        if c % 2 == 0:
                            eng, tt = nc.gpsimd, tmpg_t
                        else:
                            eng, tt = nc.vector, tmp_t
                        eng.tensor_tensor(
                            out=tt[:, lo:hi], in0=v_t[c][:, lo:hi],
                            in1=mb_t[:, lo:hi], op=mybir.AluOpType.add)
                        off = ((r - half * RH) * C + c) * SEG_PP
                        nc.vector.tensor_reduce(
                            out=out_t[:, off + jA:off + jB + 1],
                            in_=tt[:, lo:hi].rearrange("p (j k) -> p j k",
                                                       k=SEG_K),
                            axis=mybir.AxisListType.X,
                            op=mybir.AluOpType.max)
                nc.sync.dma_start(
                    out=out[:, half * RH * C * SEG_PP:(half + 1) * RH * C * SEG_PP],
                    in_=out_t[:])
    nc.compile()
    _NC = nc
    return nc


def kernel(bev_feat, bev_z_bin):
    from concourse.bass_utils import run_bass_kernel_spmd

    S = _build_static()
    nc = _build_nc()
    bev_feat = np.asarray(bev_feat, dtype=np.float32)
    bev_z_bin = np.asarray(bev_z_bin, dtype=np.int32)

    in_maps = []
    metas = []
    for core in range(8):
        b, q = core // 4, core % 4
        Sq = S.quads[q]
        flat = bev_feat[b].reshape(C, H_B * W_B)
        v = np.full((C, P * F), NEG, np.float32)
        v[:, Sq.dst] = flat[:, Sq.qpix_src]
        zflat = bev_z_bin[b, 0].reshape(H_B * W_B)
        z = np.zeros(P * F, np.float32)
        z[Sq.dst] = zflat[Sq.qpix_src].astype(np.float32)
        in_maps.append({
            "vals": v.reshape(C, P, F),
            "zb": z.reshape(P, F),
            "ltab": Sq.l_tab,
            "htab": Sq.H_tab,
        })
        metas.append((b, q))

    res = run_bass_kernel_spmd(nc, in_maps, list(range(8)))

    outp = np.zeros((B, C, H_R, W_R), np.float32)
    for core, (b, q) in enumerate(metas):
        Sq = S.quads[q]
        o = res.results[core]["out"].reshape(P, H_R, C, SEG_PP)
        # segment s lives at partition s % P, seg-slot s // P
        o = o.transpose(1, 2, 3, 0).reshape(H_R, C, NSEG)[:, :, :Sq.nseg]
        o = o[:, :, Sq.col_order]
        red = np.maximum.reduceat(o, Sq.col_starts, axis=2)
        block = np.where(red < -1.0e29, np.float32(0), red)
        # block is [H_R, C, ncols_used] -> outp[b] is [C, H_R, W_R]
        outp[b][:, :, Sq.uniq_cols] = block.transpose(1, 0, 2)
    return outp
